# revision 21
# baseline (speedup 1.0000x reference)
"""DeepTensorNeuralNetwork (DTNN / gnn_message_passing) Trainium2 kernel.

Math (per reference):
    d_sum = distance.sum(axis=2)                                  # (B,N,R)
    for l in 0..2:
        cf = x @ Wcf[l].T + bcf[l]                                # (B,N,H)
        df = d_sum @ Wdf[l].T + N*bdf[l]                          # (B,N,H)
        h  = (cf*df) @ Wfc[l].T                                   # (B,N,F)
        x  = h + tanh(h)
    g = x.sum(axis=1); out = (g @ fc0.T + b0) @ ow.T + ob         # (B,1)

Strategy: data-parallel over batch across 8 NeuronCores (8 batches each).
The dominant cost is streaming `distance` (33.5 MB/core) from HBM at
~358 GB/s (~94us floor). The j-reduction is an fp32 DVE binary fold
tree (measured: tensor_reduce is 1.64 cyc/elem, folds are 1 cyc/output;
GpSimd "help" serializes against DVE on the shared SBUF port, so DVE
does everything). Each batch's DMA is split into j-halves so folding
starts after the first half lands. The layer pipeline runs in bf16
(inputs rounded to bf16; PSUM accumulation stays fp32): matmuls process
batch GROUPS (4,2,2 - the last groups are small to shorten the tail
chain) on the PE; ACT does PSUM->SBUF bias copies and tanh; DVE does
cf*df and the residual add. The affine head (fc0 + out) is folded on
the host into a single length-F vector + scalar bias. All constants
ship in ONE packed array -> one DMA -> one wait sem.
"""

import numpy as np

B, N, F, R, H = 64, 128, 128, 64, 256
L = 3
NCORES = 8
BL = B // NCORES   # batches per core
GROUPS = (4, 2, 2)  # batches per layer-compute group

# wpack layout, fp32 columns (fp16 sections hold 2 fp16 per column):
#   [0, 384)      wcf lhsT bf16 : bf-col l*H+h           = Wcf_w[l, h, f]
#   [384, 768)    wfc lhsT bf16 : bf-col (l*2+c)*F+f     = Wfc_w[l, f, c*128+hc]
#   [768, 774)    cf bias fp32  : col l*2+c              = Wcf_b[l, c*128+h]
#   [774, 780)    df bias fp32  : col l*2+c              = N * Wdf_b[l, c*128+h]
#   [780, 781)    head lhsT bf16: bf-col 0               = (out_w @ fc0_w)[0, f]
#   [784, 1168)   wdf lhsT bf16 : bf-col l*H+h (rows<64) = Wdf_w[l, h, r]
#   [1168, 1680)  x bf16        : bf-col b*N+n           = x[b_local, n, f]
#   [1680, 1744)  identity fp16-packed
BCF_OFF = 768
BDF_OFF = 774
HEAD_OFF = 780
HEAD32_OFF = 781
WDF_OFF = 784
XOFF = 1168
IDOFF = 1680
WCOLS = 1808

_CACHE = {}


def _build_program():
    import concourse.bass as bass
    from concourse import bacc
    import concourse.tile as tile
    from concourse import mybir

    f32 = mybir.dt.float32
    bf16 = mybir.dt.float16
    AX = mybir.AxisListType
    AF = mybir.ActivationFunctionType

    nc = bacc.Bacc("TRN2")
    dist = nc.declare_dram_parameter("dist", [BL, N, N, R], f32, isOutput=False)
    wpack = nc.declare_dram_parameter("wpack", [128, WCOLS], f32, isOutput=False)
    out_ext = nc.declare_dram_parameter("out", [BL, 1], f32, isOutput=True)

    with tile.TileContext(nc) as tc:
        with (
            tc.tile_pool(name="consts", bufs=1) as consts,
            tc.tile_pool(name="dist", bufs=4) as dist_pool,
            tc.tile_pool(name="fold", bufs=2) as fold_pool,
            tc.tile_pool(name="dsum", bufs=2) as dsum_pool,
            tc.tile_pool(name="work", bufs=2) as work,
            tc.tile_pool(name="ps1", bufs=1, space="PSUM") as ps1,
            tc.tile_pool(name="ps2", bufs=2, space="PSUM") as ps2,
        ):
            # issue the first distance loads BEFORE the weight pack so the
            # fold pipeline starts as early as possible (Sync queue is FIFO)
            dist_tiles = {}

            def start_dist_dma(b):
                t = dist_pool.tile([N, N * R], f32, tag="dist")
                dflat = dist[b, :, :, :].rearrange("n j r -> n (j r)")
                nchunk = 4 if b in (0, BL - 1) else 2
                cw = 8192 // nchunk
                for k in range(nchunk):
                    nc.sync.dma_start(out=t[:, k * cw : (k + 1) * cw],
                                      in_=dflat[:, k * cw : (k + 1) * cw])
                dist_tiles[b] = t

            for b in range(3):
                start_dist_dma(b)

            wp = consts.tile([128, WCOLS], f32)
            nc.sync.dma_start(out=wp, in_=wpack[:, :])
            wb = wp.bitcast(bf16)  # (128, 2*WCOLS) bf16 view
            ident = wb[:, 2 * IDOFF : 2 * IDOFF + 128]
            out_acc = consts.tile([1, BL], f32)

            def wcf_l(l, c):
                o = l * H + c * 128
                return wb[:, o : o + 128]

            def wdf_l(l, c):
                o = 2 * WDF_OFF + l * H + c * 128
                return wb[0:R, o : o + 128]

            def wfc_l(l, c):
                o = 2 * 384 + (l * 2 + c) * F
                return wb[:, o : o + F]

            def bcf_l(l, c):
                o = BCF_OFF + l * 2 + c
                return wp[:, o : o + 1]

            def bdf_l(l, c):
                o = BDF_OFF + l * 2 + c
                return wp[:, o : o + 1]

            def chunk_fold(src, off, width, tag_suffix):
                """DVE fold tree over src[:, off:off+width] -> 64 elems/lane.
                First fold casts fp32->fp16; the rest run in 2x mode."""
                s = fold_pool.tile([N, 2048], bf16, tag=f"s{tag_suffix}")
                hw = width // 2
                nc.vector.tensor_add(s[:, 0:hw], src[:, off : off + hw],
                                     src[:, off + hw : off + width])
                t = fold_pool.tile([N, 1024], bf16, tag=f"t{tag_suffix}")
                cur, other, w = s, t, hw // 2
                while w >= 64:
                    nc.vector.tensor_add(other[:, 0:w], cur[:, 0:w], cur[:, w : 2 * w])
                    cur, other = other, cur
                    w //= 2
                return cur

            def reduce_j(b):
                dist_t = dist_tiles.pop(b)
                if b + 3 < BL:  # keep three batches in flight
                    start_dist_dma(b + 3)
                dsum = dsum_pool.tile([N, R], bf16, tag="dsum")
                if b in (0, BL - 1):  # quarter folds: earlier start / overlap
                    g0 = chunk_fold(dist_t, 0, 2048, "a")
                    g1 = chunk_fold(dist_t, 2048, 2048, "b")
                    h0 = dsum_pool.tile([N, R], bf16, tag="hsum", name="h0")
                    nc.vector.tensor_add(h0, g0[:, 0:64], g1[:, 0:64])
                    g2 = chunk_fold(dist_t, 4096, 2048, "a")
                    g3 = chunk_fold(dist_t, 6144, 2048, "b")
                    h1 = dsum_pool.tile([N, R], bf16, tag="hsum", name="h1")
                    nc.vector.tensor_add(h1, g2[:, 0:64], g3[:, 0:64])
                    nc.vector.tensor_add(dsum, h0, h1)
                else:
                    g0 = chunk_fold(dist_t, 0, 4096, "a")
                    g1 = chunk_fold(dist_t, 4096, 4096, "b")
                    nc.vector.tensor_add(dsum, g0[:, 0:64], g1[:, 0:64])
                return dsum

            b0 = 0
            for G in GROUPS:
                bs = list(range(b0, b0 + G))
                b0 += G
                NG = G * N
                # d_sum for the group's batches; transpose to (r, n); pack bf16
                dsT = dsum_pool.tile([R, 4 * N], bf16, tag="dsT")
                for k, b in enumerate(bs):
                    dsum = reduce_j(b)
                    trp = ps1.tile([R, N], bf16, tag="tr")
                    nc.tensor.transpose(trp, dsum, ident)
                    nc.scalar.activation(
                        out=dsT[:, k * N : (k + 1) * N], in_=trp, func=AF.Copy
                    )

                xc = wb[:, 2 * XOFF + bs[0] * N : 2 * XOFF + (bs[-1] + 1) * N]  # (F, NG) bf16
                for l in range(L):
                    ms = []
                    for c in range(2):
                        cfp = ps1.tile([128, 4 * N], f32, tag=f"cf{c}", name=f"cfp{c}")[:, 0:NG]
                        nc.tensor.matmul(cfp, wcf_l(l, c), xc, start=True, stop=True)
                        dfp = ps1.tile([128, 4 * N], f32, tag=f"df{c}", name=f"dfp{c}")[:, 0:NG]
                        nc.tensor.matmul(dfp, wdf_l(l, c), dsT[:, 0:NG], start=True, stop=True)
                        cfs = work.tile([128, 4 * N], bf16, tag=f"cfs{c}", name=f"cfs{c}")[:, 0:NG]
                        nc.scalar.activation(out=cfs, in_=cfp, func=AF.Identity, bias=bcf_l(l, c))
                        dfs = work.tile([128, 4 * N], bf16, tag=f"dfs{c}", name=f"dfs{c}")[:, 0:NG]
                        nc.scalar.activation(out=dfs, in_=dfp, func=AF.Identity, bias=bdf_l(l, c))
                        m = work.tile([128, 4 * N], bf16, tag=f"m{c}", name=f"m{c}")[:, 0:NG]
                        nc.vector.tensor_mul(m, cfs, dfs)
                        ms.append(m)
                    hp = ps2.tile([F, 4 * N], f32, tag="h", name="hp")[:, 0:NG]
                    nc.tensor.matmul(hp, wfc_l(l, 0), ms[0], start=True, stop=False)
                    nc.tensor.matmul(hp, wfc_l(l, 1), ms[1], start=False, stop=True)
                    th = work.tile([F, 4 * N], f32, tag="t", name="th")[:, 0:NG]
                    nc.scalar.activation(out=th, in_=hp, func=AF.Tanh)
                    xdt = f32 if l == L - 1 else bf16
                    xn = work.tile([F, 4 * N], xdt, tag=f"x{l}", name="xn")[:, 0:NG]
                    nc.vector.tensor_add(xn, hp, th)
                    xc = xn

                # head: out[b] = sum_n sum_f x[f, n] * w_head[f]
                hd = ps1.tile([1, 4 * N], f32, tag="hd", name="hd")[:, 0:NG]
                nc.tensor.matmul(hd, wp[:, HEAD32_OFF : HEAD32_OFF + 1], xc,
                                 start=True, stop=True)
                nc.vector.tensor_reduce(
                    out=out_acc[0:1, bs[0] : bs[0] + G],
                    in_=hd.rearrange("o (b n) -> o b n", b=G),
                    axis=AX.X,
                    op=mybir.AluOpType.add,
                )

            nc.sync.dma_start(out=out_ext.rearrange("b o -> o b"), in_=out_acc)

    return nc


def _host_pack(x, Wcf_w, Wcf_b, Wdf_w, Wdf_b, Wfc_w, fc0_w, fc0_b, out_w, out_b):
    import ml_dtypes

    f = np.float32
    bf = np.float16

    def pack_bf(a):  # (128, 2K) bf16 -> (128, K) fp32 bit-packed
        return np.ascontiguousarray(a.astype(bf)).view(f)

    base = np.zeros((128, WCOLS), f)
    base[:, 0:384] = pack_bf(np.asarray(Wcf_w, f).transpose(2, 0, 1).reshape(128, L * H))
    base[:, 384:768] = pack_bf(
        np.asarray(Wfc_w, f).reshape(L, F, 2, 128).transpose(3, 0, 2, 1).reshape(128, L * 2 * F)
    )
    base[:, BCF_OFF : BCF_OFF + 6] = (
        np.asarray(Wcf_b, f).reshape(L, 2, 128).transpose(2, 0, 1).reshape(128, 6)
    )
    base[:, BDF_OFF : BDF_OFF + 6] = (
        (N * np.asarray(Wdf_b, f)).reshape(L, 2, 128).transpose(2, 0, 1).reshape(128, 6)
    )
    w_head = (np.asarray(out_w, np.float64) @ np.asarray(fc0_w, np.float64))[0]  # (F,)
    head_pair = np.zeros((128, 2), f)
    head_pair[:, 0] = w_head.astype(f)
    base[:, HEAD_OFF : HEAD_OFF + 1] = pack_bf(head_pair)
    base[:, HEAD32_OFF] = w_head.astype(f)
    base[0:R, WDF_OFF : WDF_OFF + 384] = pack_bf(
        np.asarray(Wdf_w, f).transpose(2, 0, 1).reshape(R, L * H)
    )
    base[:, IDOFF : IDOFF + 64] = pack_bf(np.eye(128, dtype=f))

    b_head = float((np.asarray(out_w, np.float64) @ np.asarray(fc0_b, np.float64)
                    + np.asarray(out_b, np.float64)).reshape(()))

    x_t = np.asarray(x, f).transpose(0, 2, 1)  # (B, F, N)
    wpacks = []
    for i in range(NCORES):
        wp = base.copy()
        wp[:, XOFF : XOFF + BL * N // 2] = pack_bf(
            x_t[i * BL : (i + 1) * BL].transpose(1, 0, 2).reshape(128, BL * N)
        )
        wpacks.append(wp)
    return wpacks, b_head


def run(trace=False, **inputs):
    from concourse.bass_utils import run_bass_kernel_spmd

    distance = np.ascontiguousarray(np.asarray(inputs["distance"], np.float32))
    wpacks, b_head = _host_pack(
        inputs["x"], inputs["Wcf_w"], inputs["Wcf_b"], inputs["Wdf_w"], inputs["Wdf_b"],
        inputs["Wfc_w"], inputs["fc0_w"], inputs["fc0_b"], inputs["out_w"], inputs["out_b"],
    )

    if "nc" not in _CACHE:
        nc = _build_program()
        nc.finalize()
        _CACHE["nc"] = nc
    nc = _CACHE["nc"]

    in_maps = []
    for i in range(NCORES):
        in_maps.append({
            "dist": np.ascontiguousarray(distance[i * BL : (i + 1) * BL]),
            "wpack": wpacks[i],
        })
    res = run_bass_kernel_spmd(nc, in_maps, list(range(NCORES)), trace=trace)
    out = np.concatenate([res.results[i]["out"] for i in range(NCORES)], axis=0)
    out = (out.astype(np.float64) + b_head).astype(np.float32)
    return out, res


def kernel(**inputs):
    out, _ = run(trace=False, **inputs)
    return out


# revision 23
# speedup vs baseline: 1.0270x; 1.0270x over previous
"""DeepTensorNeuralNetwork (DTNN / gnn_message_passing) Trainium2 kernel.

Math (per reference):
    d_sum = distance.sum(axis=2)                                  # (B,N,R)
    for l in 0..2:
        cf = x @ Wcf[l].T + bcf[l]                                # (B,N,H)
        df = d_sum @ Wdf[l].T + N*bdf[l]                          # (B,N,H)
        h  = (cf*df) @ Wfc[l].T                                   # (B,N,F)
        x  = h + tanh(h)
    g = x.sum(axis=1); out = (g @ fc0.T + b0) @ ow.T + ob         # (B,1)

Strategy: data-parallel over batch across 8 NeuronCores (8 batches each).
The dominant cost is streaming `distance` (33.5 MB/core) from HBM at
~358 GB/s (~94us floor). The j-reduction is an fp32 DVE binary fold
tree (measured: tensor_reduce is 1.64 cyc/elem, folds are 1 cyc/output;
GpSimd "help" serializes against DVE on the shared SBUF port, so DVE
does everything). Each batch's DMA is split into j-halves so folding
starts after the first half lands. The layer pipeline runs in fp16
(inputs rounded to fp16; PSUM accumulation stays fp32): matmuls process
batch GROUPS (4,2,2 - the last groups are small to shorten the tail
chain) on the PE; ACT does PSUM->SBUF bias copies and tanh; DVE does
cf*df and the residual add. The affine head (fc0 + out) is folded on
the host into a single length-F vector + scalar bias. All constants
ship in ONE packed array -> one DMA -> one wait sem.
"""

import numpy as np

B, N, F, R, H = 64, 128, 128, 64, 256
L = 3
NCORES = 8
BL = B // NCORES   # batches per core
GROUPS = (4, 2, 2)  # batches per layer-compute group

# wpack layout, fp32 columns (fp16 sections hold 2 fp16 per column):
#   [0, 384)      wcf lhsT bf16 : bf-col l*H+h           = Wcf_w[l, h, f]
#   [384, 768)    wfc lhsT bf16 : bf-col (l*2+c)*F+f     = Wfc_w[l, f, c*128+hc]
#   [768, 774)    cf bias fp32  : col l*2+c              = Wcf_b[l, c*128+h]
#   [774, 780)    df bias fp32  : col l*2+c              = N * Wdf_b[l, c*128+h]
#   [780, 781)    head lhsT bf16: bf-col 0               = (out_w @ fc0_w)[0, f]
#   [784, 1168)   wdf lhsT bf16 : bf-col l*H+h (rows<64) = Wdf_w[l, h, r]
#   [1168, 1680)  x bf16        : bf-col b*N+n           = x[b_local, n, f]
#   [1680, 1744)  identity fp16-packed
BCF_OFF = 768
BDF_OFF = 774
HEAD_OFF = 780
HEAD32_OFF = 781
WDF_OFF = 784
XOFF = 1168
IDOFF = 1680
WCOLS = 1808

_CACHE = {}


def _build_program():
    import concourse.bass as bass
    from concourse import bacc
    import concourse.tile as tile
    from concourse import mybir

    f32 = mybir.dt.float32
    bf16 = mybir.dt.float16
    AX = mybir.AxisListType
    AF = mybir.ActivationFunctionType

    nc = bacc.Bacc("TRN2")
    dist = nc.declare_dram_parameter("dist", [BL, N, N, R], f32, isOutput=False)
    wpack = nc.declare_dram_parameter("wpack", [128, WCOLS], f32, isOutput=False)
    out_ext = nc.declare_dram_parameter("out", [BL, 1], f32, isOutput=True)

    with tile.TileContext(nc) as tc:
        with (
            tc.tile_pool(name="consts", bufs=1) as consts,
            tc.tile_pool(name="dist", bufs=4) as dist_pool,
            tc.tile_pool(name="fold", bufs=2) as fold_pool,
            tc.tile_pool(name="dsum", bufs=2) as dsum_pool,
            tc.tile_pool(name="work", bufs=2) as work,
            tc.tile_pool(name="ps1", bufs=1, space="PSUM") as ps1,
            tc.tile_pool(name="ps2", bufs=2, space="PSUM") as ps2,
        ):
            # issue the first distance loads BEFORE the weight pack so the
            # fold pipeline starts as early as possible (Sync queue is FIFO)
            dist_tiles = {}

            def start_dist_dma(b):
                t = dist_pool.tile([N, N * R], f32, tag="dist")
                dflat = dist[b, :, :, :].rearrange("n j r -> n (j r)")
                nchunk = 4 if b in (0, BL - 1) else 2
                cw = 8192 // nchunk
                for k in range(nchunk):
                    eng = nc.sync if (k % 2 == 0) else nc.scalar
                    eng.dma_start(out=t[:, k * cw : (k + 1) * cw],
                                  in_=dflat[:, k * cw : (k + 1) * cw])
                dist_tiles[b] = t

            for b in range(3):
                start_dist_dma(b)

            wp = consts.tile([128, WCOLS], f32)
            nc.sync.dma_start(out=wp, in_=wpack[:, :])
            wb = wp.bitcast(bf16)  # (128, 2*WCOLS) bf16 view
            ident = wb[:, 2 * IDOFF : 2 * IDOFF + 128]
            out_acc = consts.tile([1, BL], f32)

            def wcf_l(l, c):
                o = l * H + c * 128
                return wb[:, o : o + 128]

            def wdf_l(l, c):
                o = 2 * WDF_OFF + l * H + c * 128
                return wb[0:R, o : o + 128]

            def wfc_l(l, c):
                o = 2 * 384 + (l * 2 + c) * F
                return wb[:, o : o + F]

            def bcf_l(l, c):
                o = BCF_OFF + l * 2 + c
                return wp[:, o : o + 1]

            def bdf_l(l, c):
                o = BDF_OFF + l * 2 + c
                return wp[:, o : o + 1]

            def chunk_fold(src, off, width, tag_suffix):
                """DVE fold tree over src[:, off:off+width] -> 64 elems/lane.
                First fold casts fp32->fp16; the rest run in 2x mode."""
                s = fold_pool.tile([N, 2048], bf16, tag=f"s{tag_suffix}")
                hw = width // 2
                nc.vector.tensor_add(s[:, 0:hw], src[:, off : off + hw],
                                     src[:, off + hw : off + width])
                t = fold_pool.tile([N, 1024], bf16, tag=f"t{tag_suffix}")
                cur, other, w = s, t, hw // 2
                while w >= 64:
                    nc.vector.tensor_add(other[:, 0:w], cur[:, 0:w], cur[:, w : 2 * w])
                    cur, other = other, cur
                    w //= 2
                return cur

            def reduce_j(b):
                dist_t = dist_tiles.pop(b)
                if b + 3 < BL:  # keep three batches in flight
                    start_dist_dma(b + 3)
                dsum = dsum_pool.tile([N, R], bf16, tag="dsum")
                if b in (0, BL - 1):  # quarter folds: earlier start / overlap
                    g0 = chunk_fold(dist_t, 0, 2048, "a")
                    g1 = chunk_fold(dist_t, 2048, 2048, "b")
                    h0 = dsum_pool.tile([N, R], bf16, tag="hsum", name="h0")
                    nc.vector.tensor_add(h0, g0[:, 0:64], g1[:, 0:64])
                    g2 = chunk_fold(dist_t, 4096, 2048, "a")
                    g3 = chunk_fold(dist_t, 6144, 2048, "b")
                    h1 = dsum_pool.tile([N, R], bf16, tag="hsum", name="h1")
                    nc.vector.tensor_add(h1, g2[:, 0:64], g3[:, 0:64])
                    nc.vector.tensor_add(dsum, h0, h1)
                else:
                    g0 = chunk_fold(dist_t, 0, 4096, "a")
                    g1 = chunk_fold(dist_t, 4096, 4096, "b")
                    nc.vector.tensor_add(dsum, g0[:, 0:64], g1[:, 0:64])
                return dsum

            b0 = 0
            for G in GROUPS:
                bs = list(range(b0, b0 + G))
                b0 += G
                NG = G * N
                # d_sum for the group's batches; transpose to (r, n); pack fp16
                dsT = dsum_pool.tile([R, 4 * N], bf16, tag="dsT")
                for k, b in enumerate(bs):
                    dsum = reduce_j(b)
                    trp = ps1.tile([R, N], bf16, tag="tr")
                    nc.tensor.transpose(trp, dsum, ident)
                    nc.scalar.activation(
                        out=dsT[:, k * N : (k + 1) * N], in_=trp, func=AF.Copy
                    )

                xc = wb[:, 2 * XOFF + bs[0] * N : 2 * XOFF + (bs[-1] + 1) * N]  # (F, NG) bf16
                for l in range(L):
                    ms = []
                    for c in range(2):
                        cfp = ps1.tile([128, 4 * N], f32, tag=f"cf{c}", name=f"cfp{c}")[:, 0:NG]
                        nc.tensor.matmul(cfp, wcf_l(l, c), xc, start=True, stop=True)
                        dfp = ps1.tile([128, 4 * N], f32, tag=f"df{c}", name=f"dfp{c}")[:, 0:NG]
                        nc.tensor.matmul(dfp, wdf_l(l, c), dsT[:, 0:NG], start=True, stop=True)
                        cfs = work.tile([128, 4 * N], bf16, tag=f"cfs{c}", name=f"cfs{c}")[:, 0:NG]
                        nc.scalar.activation(out=cfs, in_=cfp, func=AF.Identity, bias=bcf_l(l, c))
                        dfs = work.tile([128, 4 * N], bf16, tag=f"dfs{c}", name=f"dfs{c}")[:, 0:NG]
                        nc.scalar.activation(out=dfs, in_=dfp, func=AF.Identity, bias=bdf_l(l, c))
                        m = work.tile([128, 4 * N], bf16, tag=f"m{c}", name=f"m{c}")[:, 0:NG]
                        nc.vector.tensor_mul(m, cfs, dfs)
                        ms.append(m)
                    hp = ps2.tile([F, 4 * N], f32, tag="h", name="hp")[:, 0:NG]
                    nc.tensor.matmul(hp, wfc_l(l, 0), ms[0], start=True, stop=False)
                    nc.tensor.matmul(hp, wfc_l(l, 1), ms[1], start=False, stop=True)
                    th = work.tile([F, 4 * N], f32, tag="t", name="th")[:, 0:NG]
                    nc.scalar.activation(out=th, in_=hp, func=AF.Tanh)
                    xdt = f32 if l == L - 1 else bf16
                    xn = work.tile([F, 4 * N], xdt, tag=f"x{l}", name="xn")[:, 0:NG]
                    nc.vector.tensor_add(xn, hp, th)
                    xc = xn

                # head: out[b] = sum_n sum_f x[f, n] * w_head[f]
                hd = ps1.tile([1, 4 * N], f32, tag="hd", name="hd")[:, 0:NG]
                nc.tensor.matmul(hd, wp[:, HEAD32_OFF : HEAD32_OFF + 1], xc,
                                 start=True, stop=True)
                nc.vector.tensor_reduce(
                    out=out_acc[0:1, bs[0] : bs[0] + G],
                    in_=hd.rearrange("o (b n) -> o b n", b=G),
                    axis=AX.X,
                    op=mybir.AluOpType.add,
                )

            nc.sync.dma_start(out=out_ext.rearrange("b o -> o b"), in_=out_acc)

    return nc


def _host_pack(x, Wcf_w, Wcf_b, Wdf_w, Wdf_b, Wfc_w, fc0_w, fc0_b, out_w, out_b):
    import ml_dtypes

    f = np.float32
    bf = np.float16

    def pack_bf(a):  # (128, 2K) bf16 -> (128, K) fp32 bit-packed
        return np.ascontiguousarray(a.astype(bf)).view(f)

    base = np.zeros((128, WCOLS), f)
    base[:, 0:384] = pack_bf(np.asarray(Wcf_w, f).transpose(2, 0, 1).reshape(128, L * H))
    base[:, 384:768] = pack_bf(
        np.asarray(Wfc_w, f).reshape(L, F, 2, 128).transpose(3, 0, 2, 1).reshape(128, L * 2 * F)
    )
    base[:, BCF_OFF : BCF_OFF + 6] = (
        np.asarray(Wcf_b, f).reshape(L, 2, 128).transpose(2, 0, 1).reshape(128, 6)
    )
    base[:, BDF_OFF : BDF_OFF + 6] = (
        (N * np.asarray(Wdf_b, f)).reshape(L, 2, 128).transpose(2, 0, 1).reshape(128, 6)
    )
    w_head = (np.asarray(out_w, np.float64) @ np.asarray(fc0_w, np.float64))[0]  # (F,)
    head_pair = np.zeros((128, 2), f)
    head_pair[:, 0] = w_head.astype(f)
    base[:, HEAD_OFF : HEAD_OFF + 1] = pack_bf(head_pair)
    base[:, HEAD32_OFF] = w_head.astype(f)
    base[0:R, WDF_OFF : WDF_OFF + 384] = pack_bf(
        np.asarray(Wdf_w, f).transpose(2, 0, 1).reshape(R, L * H)
    )
    base[:, IDOFF : IDOFF + 64] = pack_bf(np.eye(128, dtype=f))

    b_head = float((np.asarray(out_w, np.float64) @ np.asarray(fc0_b, np.float64)
                    + np.asarray(out_b, np.float64)).reshape(()))

    x_t = np.asarray(x, f).transpose(0, 2, 1)  # (B, F, N)
    wpacks = []
    for i in range(NCORES):
        wp = base.copy()
        wp[:, XOFF : XOFF + BL * N // 2] = pack_bf(
            x_t[i * BL : (i + 1) * BL].transpose(1, 0, 2).reshape(128, BL * N)
        )
        wpacks.append(wp)
    return wpacks, b_head


def run(trace=False, **inputs):
    from concourse.bass_utils import run_bass_kernel_spmd

    distance = np.ascontiguousarray(np.asarray(inputs["distance"], np.float32))
    wpacks, b_head = _host_pack(
        inputs["x"], inputs["Wcf_w"], inputs["Wcf_b"], inputs["Wdf_w"], inputs["Wdf_b"],
        inputs["Wfc_w"], inputs["fc0_w"], inputs["fc0_b"], inputs["out_w"], inputs["out_b"],
    )

    if "nc" not in _CACHE:
        nc = _build_program()
        nc.finalize()
        _CACHE["nc"] = nc
    nc = _CACHE["nc"]

    in_maps = []
    for i in range(NCORES):
        in_maps.append({
            "dist": np.ascontiguousarray(distance[i * BL : (i + 1) * BL]),
            "wpack": wpacks[i],
        })
    res = run_bass_kernel_spmd(nc, in_maps, list(range(NCORES)), trace=trace)
    out = np.concatenate([res.results[i]["out"] for i in range(NCORES)], axis=0)
    out = (out.astype(np.float64) + b_head).astype(np.float32)
    return out, res


def kernel(**inputs):
    out, _ = run(trace=False, **inputs)
    return out
